# revision 5
# baseline (speedup 1.0000x reference)
"""Trainium2 Bass kernel for nn_BoundaryLoss (boundary EDT + weighted L1 loss).

Strategy (pure data parallel, 1 image per NeuronCore, 8 cores):
  Per image on device:
    comp    = target <= 0.5                      (complement of binary mask)
    dil     = dilate3x3(comp)  (horiz in row-major, vert after PE transpose)
    boundT  = dil - comp  (= binary AND dilated-complement, transposed)
    d2      = windowed exact EDT: vertical L1 via log-doubling (window 3),
              horizontal parabola min over |u| <= 3
    outputs per partition: sum(|sqrt(d2) * (sigmoid(pred)-target)|), max(d2)
  Host: final 128-way reduction per image + normalization + batch mean.

Windowed EDT exactness: windowed d2 >= true d2 always, with equality
guaranteed when max(windowed d2) <= KH^2 (KH = 3).  The device returns
max(d2); the host verifies the bound and falls back to an exact numpy
path for any image that fails it (never on dense masks).

vs the previous version:
  - no identity DMA: ident generated on-chip (gpsimd iota + is_eq) in the
    dead time before the input lands
  - pred queued BEHIND tgt on the same Sync HWDGE queue, so the critical
    tgt DMA has the DMA engines to itself (~1.9us earlier compute start)
  - b01/is_gt dropped: boundT = dT - nbpT (dilation minus complement)
  - all 8 stage-A transposes land in ONE 2KB PSUM bank; one ACT copy
    moves the 4 nbp blocks, one DVE copy moves the 4 dr blocks
    (replaces 8 separate per-block copies); same single-copy trick for
    the 4 g2 blocks
  - bf16 diff/dist/junk tail (dist*diff in bf16, |.| folded into reduce)
"""

import os
from contextlib import ExitStack

import numpy as np

H = 256
W = 256
P = 128
C = 2  # partition chunks per image (H = C * P)
KH = 3  # phase-2 horizontal window (exactness proof bound: m2 <= KH*KH)
BIGF = 16384.0  # phase-1 sentinel (bf16-exact; BIGF + small stays BIGF in bf16)
BIG2 = 3.0e8  # phase-2 border pad, > (BIGF+3)^2
PAD1 = 4  # phase-1 doubling pads (window 1+2; 4 keeps slices 4B-aligned)
FW = H + 2 * PAD1
GW = W + 2 * KH

LAST_RESULTS = None  # BassKernelResults of the most recent device run


def _build_nc():
    import concourse.bass as bass
    import concourse.mybir as mybir

    bf16 = mybir.dt.bfloat16
    f32 = mybir.dt.float32
    i32 = mybir.dt.int32
    Alu = mybir.AluOpType
    Act = mybir.ActivationFunctionType

    nc = bass.Bass(detect_race_conditions=False)
    inp_d = nc.dram_tensor("inp", [P, 4 * W], f32, kind="ExternalInput")
    out_d = nc.dram_tensor("out", [P, 4], f32, kind="ExternalOutput")

    ctx = ExitStack()
    sb = lambda name, shape, dt: ctx.enter_context(nc.sbuf_tensor(name, shape, dt))

    with ctx:
        inp = sb("inp_t", [P, 4, W], f32)
        tgt = inp[:, 0:C, :]
        prd = inp[:, C : 2 * C, :]
        ident = sb("ident", [P, P], bf16)
        iot = sb("iot", [P, P], i32)
        nbp = sb("nbp", [P, C, W + 2], bf16)
        t1 = sb("t1", [P, C, W], bf16)
        dr = sb("dr", [P, C, W], bf16)
        bT = sb("bT", [P, C, H], bf16)       # transposed complement
        drTp = sb("drTp", [P, C, H + 2], bf16)
        t2 = sb("t2", [P, C, H], bf16)
        dT = sb("dT", [P, C, H], bf16)
        boundT = sb("boundT", [P, C, H], bf16)
        fvA = sb("fvA", [P, C, FW], bf16)
        fvB = sb("fvB", [P, C, FW], bf16)
        fvC = sb("fvC", [P, C, FW], bf16)
        tmpd = sb("tmpd", [P, C, FW], bf16)
        g2T = sb("g2T", [P, C, H], bf16)
        g2p = sb("g2p", [P, C, GW], bf16)
        p2tmp = sb("p2tmp", [P, C, W], bf16)
        p2acc = [sb(f"p2acc{i}", [P, C, W], bf16) for i in range(KH)]
        dist = sb("dist", [P, C, W], bf16)
        sg = sb("sg", [P, C, W], f32)
        diff = sb("diff", [P, C, W], bf16)
        junk = sb("junk", [P, C, W], bf16)
        outb = sb("outb", [P, 4], f32)
        warm = sb("warm", [P, 4], f32)
        ps_n = ctx.enter_context(nc.psum_tensor("ps_n", [P, 4, P], bf16))
        ps_d = ctx.enter_context(nc.psum_tensor("ps_d", [P, 4, P], bf16))
        psg = ctx.enter_context(nc.psum_tensor("psg", [P, 4, P], bf16))

        dma_sem = ctx.enter_context(nc.semaphore("dma_sem"))
        dma2_sem = ctx.enter_context(nc.semaphore("dma2_sem"))
        dve_sem = ctx.enter_context(nc.semaphore("dve_sem"))
        act_sem = ctx.enter_context(nc.semaphore("act_sem"))
        pe_sem = ctx.enter_context(nc.semaphore("pe_sem"))
        gp_sem = ctx.enter_context(nc.semaphore("gp_sem"))
        w_sem = ctx.enter_context(nc.semaphore("w_sem"))

        block = ctx.enter_context(nc.Block(no_gpsimd_drain=True))

        @block.sync
        def _(sync: "bass.BassEngine"):
            # tgt first (critical), pred queued right behind on the same queue
            sync.dma_start(out=inp[:, 0:C, :], in_=inp_d[:, 0 : C * W]).then_inc(dma_sem, 16)
            sync.dma_start(out=inp[:, C : 2 * C, :], in_=inp_d[:, C * W : 2 * C * W]).then_inc(dma2_sem, 16)
            # out DMA (after the DVE chain fully wrote outb)
            sync.wait_ge(dve_sem, 6)
            sync.dma_start(out=out_d[:], in_=outb[:]).then_inc(dma_sem, 16)
            sync.wait_ge(dma_sem, 32)

        @block.gpsimd
        def _(gpsimd: "bass.BassEngine"):
            # identity matrix on-chip: iota(j - p) == 0
            nc.gpsimd.iota(iot[:], [[1, P]], base=0, channel_multiplier=-1)
            nc.gpsimd.tensor_scalar(ident[:], iot[:], 0, None, Alu.is_equal).then_inc(gp_sem, 1)

        @block.scalar
        def _(scalar: "bass.BassEngine"):
            # warm the sigmoid/copy table while the input DMA flies
            scalar.wait_ge(w_sem, 1)
            nc.scalar.activation(warm[:, 1:2], warm[:, 0:1], Act.Sigmoid)
            nc.scalar.copy(warm[:, 1:2], warm[:, 0:1])
            # bT: all 4 nbp transpose blocks in one copy from PSUM bank
            scalar.wait_ge(pe_sem, 4)
            nc.scalar.copy(bT[:], ps_n[:, :, :]).then_inc(act_sem, 1)  # a=1
            # sigmoid (pred lands right behind tgt on the sync queue)
            scalar.wait_ge(dma2_sem, 16)
            nc.scalar.activation(sg[:], prd, Act.Sigmoid).then_inc(act_sem, 1)  # a=2
            # re-warm the sqrt table during phase 2 (sigmoid load evicted it)
            scalar.wait_ge(dve_sem, 4)
            nc.scalar.sqrt(warm[:, 2:3], warm[:, 0:1])
            # sqrt(d2) (after DVE phase 2: d=5)
            scalar.wait_ge(dve_sem, 5)
            nc.scalar.sqrt(dist[:], p2acc[KH - 1][:]).then_inc(act_sem, 1)  # a=3

        @block.tensor
        def _(tensor: "bass.BassEngine"):
            # p-state warm: dummy transposes (garbage data into psg, which the
            # real g2 transposes overwrite later) keep the PE clock ramping
            # toward full speed before the real transposes issue
            for k in range(11):
                nc.tensor.transpose(psg[:, k % 4, :], nbp[:, 0, 1 : 1 + P], warm[:, 0:1].to_broadcast([P, P]) if False else nbp[:, 1, 1 : 1 + P])
            # nbp transposes (DVE d=1) using on-chip ident (gp=1)
            tensor.wait_ge(gp_sem, 1)
            tensor.wait_ge(dve_sem, 1)
            for wb in range(C):
                for hc in range(C):
                    nc.tensor.transpose(
                        ps_n[:, wb * C + hc, :],
                        nbp[:, hc, 1 + wb * P : 1 + (wb + 1) * P],
                        ident[:],
                    ).then_inc(pe_sem, 1)  # pe 1..4
            # dr transposes (DVE d=2)
            tensor.wait_ge(dve_sem, 2)
            for wb in range(C):
                for hc in range(C):
                    nc.tensor.transpose(
                        ps_d[:, wb * C + hc, :],
                        dr[:, hc, wb * P : (wb + 1) * P],
                        ident[:],
                    ).then_inc(pe_sem, 1)  # pe 5..8
            # hold the PE p-state during the doubling (psg overwritten below)
            for k in range(11):
                nc.tensor.transpose(psg[:, k % 4, :], nbp[:, 0, 1 : 1 + P], nbp[:, 1, 1 : 1 + P])
            # g2 transposes back to row-major, per W-chunk as squares land
            for wb in range(C):
                tensor.wait_ge(dve_sem, 3 + wb)
                for hc in range(C):
                    nc.tensor.transpose(
                        psg[:, hc * C + wb, :],
                        g2T[:, wb, hc * P : (hc + 1) * P],
                        ident[:],
                    ).then_inc(pe_sem, 1)  # pe 9..12

        @block.vector
        def _(vector: "bass.BassEngine"):
            # data-independent pad memsets first (no waits)
            nc.vector.memset(nbp[:, :, 0:1], 0.0)
            nc.vector.memset(nbp[:, :, W + 1 : W + 2], 0.0)
            nc.vector.memset(drTp[:, :, 0:1], 0.0)
            nc.vector.memset(drTp[:, :, H + 1 : H + 2], 0.0)
            nc.vector.memset(fvA[:, :, 0:PAD1], BIGF)
            nc.vector.memset(fvA[:, :, PAD1 + H : FW], BIGF)
            nc.vector.memset(fvB[:, :, 0:1], BIGF)
            nc.vector.memset(fvB[:, :, FW - 1 : FW], BIGF)
            nc.vector.memset(g2p[:, :, 0:KH], BIG2)
            nc.vector.memset(g2p[:, :, KH + W : GW], BIG2)
            nc.vector.memset(outb[:, 3:4], 0.0)
            nc.vector.memset(warm[:, 0:1], 1.0).then_inc(w_sem, 1)

            vector.wait_ge(dma_sem, 16)
            # complement (padded); horizontal dilation
            nc.vector.tensor_scalar(nbp[:, :, 1 : W + 1], tgt, 0.5, None, Alu.is_le).then_inc(dve_sem, 1)  # d=1
            nc.vector.tensor_tensor(t1[:], nbp[:, :, 0:W], nbp[:, :, 2 : W + 2], Alu.max)
            nc.vector.tensor_tensor(dr[:], t1[:], nbp[:, :, 1 : W + 1], Alu.max).then_inc(dve_sem, 1)  # d=2

            # dr transpose blocks: copy wb0 pair as soon as it lands
            vector.wait_ge(pe_sem, 6)
            nc.vector.tensor_copy(drTp[:, 0, 1 : H + 1], ps_d[:, 0:2, :])
            vector.wait_ge(pe_sem, 8)
            nc.vector.tensor_copy(drTp[:, 1, 1 : H + 1], ps_d[:, 2:4, :])
            # vertical dilation + boundary (bound = dilated - complement)
            nc.vector.tensor_tensor(t2[:], drTp[:, :, 0:H], drTp[:, :, 2 : H + 2], Alu.max)
            nc.vector.tensor_tensor(dT[:], t2[:], drTp[:, :, 1 : H + 1], Alu.max)
            vector.wait_ge(act_sem, 1)
            nc.vector.tensor_tensor(boundT[:], dT[:], bT[:], Alu.subtract)
            nc.vector.tensor_scalar(
                fvA[:, :, PAD1 : PAD1 + H], boundT[:], -BIGF, BIGF, Alu.mult, Alu.add
            )
            # vertical L1 distance by log-doubling (window 1+2 = 3)
            nc.vector.tensor_tensor(
                tmpd[:, :, 1 : FW - 1], fvA[:, :, 0 : FW - 2], fvA[:, :, 2:FW], Alu.min
            )
            nc.vector.scalar_tensor_tensor(
                out=fvB[:, :, 1 : FW - 1], in0=tmpd[:, :, 1 : FW - 1], scalar=1.0,
                in1=fvA[:, :, 1 : FW - 1], op0=Alu.add, op1=Alu.min,
            )
            nc.vector.tensor_tensor(
                tmpd[:, :, 2 : FW - 2], fvB[:, :, 0 : FW - 4], fvB[:, :, 4:FW], Alu.min
            )
            nc.vector.scalar_tensor_tensor(
                out=fvC[:, :, 2 : FW - 2], in0=tmpd[:, :, 2 : FW - 2], scalar=2.0,
                in1=fvB[:, :, 2 : FW - 2], op0=Alu.add, op1=Alu.min,
            )
            # square the vertical distance, per W-chunk so PE can start early
            for wb in range(C):
                nc.vector.tensor_tensor(
                    g2T[:, wb, :],
                    fvC[:, wb, PAD1 : PAD1 + H],
                    fvC[:, wb, PAD1 : PAD1 + H],
                    Alu.mult,
                ).then_inc(dve_sem, 1)  # d=3 (wb=0), d=4 (wb=1)
            # diff in the g2-transpose gap (sigmoid ready: a>=2)
            vector.wait_ge(act_sem, 2)
            nc.vector.tensor_tensor(diff[:], sg[:], tgt, Alu.subtract)

            # single combined copy of the 4 g2 transpose blocks
            vector.wait_ge(pe_sem, 12)
            nc.vector.tensor_copy(g2p[:, :, KH : KH + W], psg[:, :, :])
            # phase 2: parabola min over |u| <= 3
            prev = None
            for u in range(1, KH + 1):
                in0 = g2p[:, :, KH - u : KH - u + W]
                in1 = g2p[:, :, KH + u : KH + u + W]
                nc.vector.tensor_tensor(p2tmp[:], in0, in1, Alu.min)
                base = g2p[:, :, KH : KH + W] if prev is None else prev[:]
                ins = nc.vector.scalar_tensor_tensor(
                    out=p2acc[u - 1][:], in0=p2tmp[:], scalar=float(u * u), in1=base,
                    op0=Alu.add, op1=Alu.min,
                )
                prev = p2acc[u - 1]
            d2 = prev
            ins.then_inc(dve_sem, 1)  # d=5 (d2 ready for ACT sqrt)
            nc.vector.tensor_reduce(
                out=outb[:, 1:3], in_=d2[:], axis=mybir.AxisListType.X, op=Alu.max
            )
            # weighted L1: |dist*diff| summed (dist >= 0)
            vector.wait_ge(act_sem, 3)
            nc.vector.tensor_tensor(junk[:], dist[:], diff[:], Alu.mult)
            nc.vector.tensor_reduce(
                out=outb[:, 0:1], in_=junk[:], axis=mybir.AxisListType.XY, op=Alu.add,
                apply_absolute_value=True,
            ).then_inc(dve_sem, 1)  # d=6 (outb complete)

    return nc


_NC_CACHE = {}


def _get_nc():
    if "nc" not in _NC_CACHE:
        _NC_CACHE["nc"] = _build_nc()
    return _NC_CACHE["nc"]


def _pack_input(tgt_i, prd_i):
    # [P, 4*W]: per partition p -> tgt rows p, p+128; pred rows p, p+128
    return np.concatenate([tgt_i[:P], tgt_i[P:], prd_i[:P], prd_i[P:]], axis=1)


# ---------- exact numpy fallback (pathological images only) ----------

def _reference_image_np(t, p):
    """Exact replica of the jax reference for one image, in numpy fp32."""
    b = (t > 0.5).astype(np.float32)
    if not (b > 0).any():
        return 0.0
    v = b.copy()
    v[1:] = np.minimum(v[1:], b[:-1])
    v[:-1] = np.minimum(v[:-1], b[1:])
    er = v.copy()
    er[:, 1:] = np.minimum(er[:, 1:], v[:, :-1])
    er[:, :-1] = np.minimum(er[:, :-1], v[:, 1:])
    bound = b - er
    if bound.sum() == 0:
        bound = b
    feat = bound > 0.5
    BIGV = np.float32(1e6)
    c = np.full(W, BIGV, np.float32)
    d_fwd = np.empty((H, W), np.float32)
    for i in range(H):
        c = np.where(feat[i], np.float32(0.0), c + 1)
        d_fwd[i] = c
    c = np.full(W, BIGV, np.float32)
    d_bwd = np.empty((H, W), np.float32)
    for i in range(H - 1, -1, -1):
        c = np.where(feat[i], np.float32(0.0), c + 1)
        d_bwd[i] = c
    g = np.minimum(d_fwd, d_bwd)
    j = np.arange(W, dtype=np.float32)
    d2 = np.empty((H, W), np.float32)
    for i in range(H):
        d2[i] = np.min(g[i][None, :] ** 2 + (j[:, None] - j[None, :]) ** 2, axis=-1)
    dist = np.sqrt(d2)
    m = dist.max()
    if m > 0:
        dist = dist / (m + np.float32(1e-8))
    sgm = 1.0 / (1.0 + np.exp(-p.astype(np.float64)))
    return float(np.mean(dist * np.abs(sgm - t)))


def _bound_empty(t):
    """True if erosion removes every boundary pixel (reference falls back)."""
    b = (t > 0.5).astype(np.float32)
    v = b.copy()
    v[1:] = np.minimum(v[1:], b[:-1])
    v[:-1] = np.minimum(v[:-1], b[1:])
    er = v.copy()
    er[:, 1:] = np.minimum(er[:, 1:], v[:, :-1])
    er[:, :-1] = np.minimum(er[:, :-1], v[:, 1:])
    return (b - er).sum() == 0


# ---------- public entry point ----------

def kernel(pred_logits: np.ndarray, target: np.ndarray) -> np.ndarray:
    global LAST_RESULTS
    from concourse.bass_utils import run_bass_kernel_spmd

    pred = np.ascontiguousarray(np.asarray(pred_logits, np.float32)[:, 0])
    tgt = np.ascontiguousarray(np.asarray(target, np.float32)[:, 0])
    B = pred.shape[0]
    assert pred.shape == (B, H, W) and tgt.shape == (B, H, W)
    assert B == 8, f"kernel is built for batch 8, got {B}"

    nc = _get_nc()
    in_maps = [{"inp": _pack_input(tgt[i], pred[i])} for i in range(B)]
    trace = bool(int(os.environ.get("KERNEL_TRACE", "0")))
    res = run_bass_kernel_spmd(nc, in_maps, core_ids=list(range(B)), trace=trace)
    LAST_RESULTS = res

    total = 0.0
    for i in range(B):
        o = np.asarray(res.results[i]["out"], np.float32)  # [128, 4]
        if not (tgt[i] > 0.5).any():
            continue  # empty mask: reference skips (loss 0)
        m2 = float(o[:, 1:3].max())
        if m2 > float(KH * KH) or _bound_empty(tgt[i]):
            # windowed EDT not provably exact for this image -> exact path
            total += _reference_image_np(tgt[i], pred[i])
            continue
        S = float(o[:, 0].sum(dtype=np.float64))
        m = np.float32(np.sqrt(np.float32(m2)))
        denom = float(m + np.float32(1e-8)) if m > 0 else 1.0
        total += (S / denom) / float(H * W)
    return np.float32(total / max(B, 1))


# revision 6
# speedup vs baseline: 1.0010x; 1.0010x over previous
"""Trainium2 Bass kernel for nn_BoundaryLoss (boundary EDT + weighted L1 loss).

Strategy (pure data parallel, 1 image per NeuronCore, 8 cores):
  Per image on device:
    comp    = target <= 0.5                      (complement of binary mask)
    dil     = dilate3x3(comp)  (horiz in row-major, vert after PE transpose)
    boundT  = dil - comp  (= binary AND dilated-complement, transposed)
    d2      = windowed exact EDT: vertical L1 via log-doubling (window 3),
              horizontal parabola min over |u| <= 3
    outputs per partition: sum(|sqrt(d2) * (sigmoid(pred)-target)|), max(d2)
  Host: final 128-way reduction per image + normalization + batch mean.

Windowed EDT exactness: windowed d2 >= true d2 always, with equality
guaranteed when max(windowed d2) <= KH^2 (KH = 3).  The device returns
max(d2); the host verifies the bound and falls back to an exact numpy
path for any image that fails it (never on dense masks).

vs the previous version:
  - no identity DMA: ident generated on-chip (gpsimd iota + is_eq) in the
    dead time before the input lands
  - pred queued BEHIND tgt on the same Sync HWDGE queue, so the critical
    tgt DMA has the DMA engines to itself (~1.9us earlier compute start)
  - b01/is_gt dropped: boundT = dT - nbpT (dilation minus complement)
  - all 8 stage-A transposes land in ONE 2KB PSUM bank; one ACT copy
    moves the 4 nbp blocks, one DVE copy moves the 4 dr blocks
    (replaces 8 separate per-block copies); same single-copy trick for
    the 4 g2 blocks
  - bf16 diff/dist/junk tail (dist*diff in bf16, |.| folded into reduce)
"""

import os
from contextlib import ExitStack

import numpy as np

H = 256
W = 256
P = 128
C = 2  # partition chunks per image (H = C * P)
KH = 3  # phase-2 horizontal window (exactness proof bound: m2 <= KH*KH)
BIGF = 16384.0  # phase-1 sentinel (bf16-exact; BIGF + small stays BIGF in bf16)
BIG2 = 3.0e8  # phase-2 border pad, > (BIGF+3)^2
PAD1 = 4  # phase-1 doubling pads (window 1+2; 4 keeps slices 4B-aligned)
FW = H + 2 * PAD1
GW = W + 2 * KH

LAST_RESULTS = None  # BassKernelResults of the most recent device run


def _build_nc():
    import concourse.bass as bass
    import concourse.mybir as mybir

    bf16 = mybir.dt.bfloat16
    f32 = mybir.dt.float32
    i32 = mybir.dt.int32
    Alu = mybir.AluOpType
    Act = mybir.ActivationFunctionType

    nc = bass.Bass(detect_race_conditions=False)
    inp_d = nc.dram_tensor("inp", [P, 4 * W], f32, kind="ExternalInput")
    out_d = nc.dram_tensor("out", [P, 4], f32, kind="ExternalOutput")

    ctx = ExitStack()
    sb = lambda name, shape, dt: ctx.enter_context(nc.sbuf_tensor(name, shape, dt))

    with ctx:
        inp = sb("inp_t", [P, 4, W], f32)
        tgt = inp[:, 0:C, :]
        prd = inp[:, C : 2 * C, :]
        ident = sb("ident", [P, P], bf16)
        iot = sb("iot", [P, P], i32)
        nbp = sb("nbp", [P, C, W + 2], bf16)
        t1 = sb("t1", [P, C, W], bf16)
        dr = sb("dr", [P, C, W], bf16)
        bT = sb("bT", [P, C, H], bf16)       # transposed complement
        drTp = sb("drTp", [P, C, H + 2], bf16)
        t2 = sb("t2", [P, C, H], bf16)
        dT = sb("dT", [P, C, H], bf16)
        boundT = sb("boundT", [P, C, H], bf16)
        fvA = sb("fvA", [P, C, FW], bf16)
        fvB = sb("fvB", [P, C, FW], bf16)
        fvC = sb("fvC", [P, C, FW], bf16)
        tmpd = sb("tmpd", [P, C, FW], bf16)
        g2T = sb("g2T", [P, C, H], bf16)
        g2p = sb("g2p", [P, C, GW], bf16)
        p2tmp = sb("p2tmp", [P, C, W], bf16)
        p2acc = [sb(f"p2acc{i}", [P, C, W], bf16) for i in range(KH)]
        dist = sb("dist", [P, C, W], bf16)
        sg = sb("sg", [P, C, W], f32)
        diff = sb("diff", [P, C, W], bf16)
        junk = sb("junk", [P, C, W], bf16)
        outb = sb("outb", [P, 4], f32)
        warm = sb("warm", [P, 4], f32)
        ps_n = ctx.enter_context(nc.psum_tensor("ps_n", [P, 4, P], bf16))
        ps_d = ctx.enter_context(nc.psum_tensor("ps_d", [P, 4, P], bf16))
        psg = ctx.enter_context(nc.psum_tensor("psg", [P, 4, P], bf16))

        dma_sem = ctx.enter_context(nc.semaphore("dma_sem"))
        dma2_sem = ctx.enter_context(nc.semaphore("dma2_sem"))
        dve_sem = ctx.enter_context(nc.semaphore("dve_sem"))
        act_sem = ctx.enter_context(nc.semaphore("act_sem"))
        pe_sem = ctx.enter_context(nc.semaphore("pe_sem"))
        gp_sem = ctx.enter_context(nc.semaphore("gp_sem"))
        w_sem = ctx.enter_context(nc.semaphore("w_sem"))

        block = ctx.enter_context(nc.Block(no_gpsimd_drain=True))

        @block.sync
        def _(sync: "bass.BassEngine"):
            # tgt first (critical), pred queued right behind on the same queue
            sync.dma_start(out=inp[:, 0:C, :], in_=inp_d[:, 0 : C * W]).then_inc(dma_sem, 16)
            sync.dma_start(out=inp[:, C : 2 * C, :], in_=inp_d[:, C * W : 2 * C * W]).then_inc(dma2_sem, 16)
            # out DMA (after the DVE chain fully wrote outb)
            sync.wait_ge(dve_sem, 6)
            sync.dma_start(out=out_d[:], in_=outb[:]).then_inc(dma_sem, 16)
            sync.wait_ge(dma_sem, 32)

        @block.gpsimd
        def _(gpsimd: "bass.BassEngine"):
            # identity matrix on-chip: iota(j - p) == 0
            nc.gpsimd.iota(iot[:], [[1, P]], base=0, channel_multiplier=-1)
            nc.gpsimd.tensor_scalar(ident[:], iot[:], 0, None, Alu.is_equal).then_inc(gp_sem, 1)

        @block.scalar
        def _(scalar: "bass.BassEngine"):
            # warm the sigmoid/copy table while the input DMA flies
            scalar.wait_ge(w_sem, 1)
            nc.scalar.activation(warm[:, 1:2], warm[:, 0:1], Act.Sigmoid)
            nc.scalar.copy(warm[:, 1:2], warm[:, 0:1])
            # bT: all 4 nbp transpose blocks in one copy from PSUM bank
            scalar.wait_ge(pe_sem, 4)
            nc.scalar.copy(bT[:], ps_n[:, :, :]).then_inc(act_sem, 1)  # a=1
            # sigmoid (pred lands right behind tgt on the sync queue)
            scalar.wait_ge(dma2_sem, 16)
            nc.scalar.activation(sg[:], prd, Act.Sigmoid).then_inc(act_sem, 1)  # a=2
            # re-warm the sqrt table during phase 2 (sigmoid load evicted it)
            scalar.wait_ge(dve_sem, 4)
            nc.scalar.sqrt(warm[:, 2:3], warm[:, 0:1])
            # sqrt(d2) (after DVE phase 2: d=5)
            scalar.wait_ge(dve_sem, 5)
            nc.scalar.sqrt(dist[:], p2acc[KH - 1][:]).then_inc(act_sem, 1)  # a=3

        @block.tensor
        def _(tensor: "bass.BassEngine"):
            # nbp transposes (DVE d=1) using on-chip ident (gp=1)
            tensor.wait_ge(gp_sem, 1)
            tensor.wait_ge(dve_sem, 1)
            for wb in range(C):
                for hc in range(C):
                    nc.tensor.transpose(
                        ps_n[:, wb * C + hc, :],
                        nbp[:, hc, 1 + wb * P : 1 + (wb + 1) * P],
                        ident[:],
                    ).then_inc(pe_sem, 1)  # pe 1..4
            # dr transposes (DVE d=2)
            tensor.wait_ge(dve_sem, 2)
            for wb in range(C):
                for hc in range(C):
                    nc.tensor.transpose(
                        ps_d[:, wb * C + hc, :],
                        dr[:, hc, wb * P : (wb + 1) * P],
                        ident[:],
                    ).then_inc(pe_sem, 1)  # pe 5..8
            # g2 transposes back to row-major, per W-chunk as squares land
            for wb in range(C):
                tensor.wait_ge(dve_sem, 3 + wb)
                for hc in range(C):
                    nc.tensor.transpose(
                        psg[:, hc * C + wb, :],
                        g2T[:, wb, hc * P : (hc + 1) * P],
                        ident[:],
                    ).then_inc(pe_sem, 1)  # pe 9..12

        @block.vector
        def _(vector: "bass.BassEngine"):
            # data-independent pad memsets first (no waits)
            nc.vector.memset(nbp[:, :, 0:1], 0.0)
            nc.vector.memset(nbp[:, :, W + 1 : W + 2], 0.0)
            nc.vector.memset(drTp[:, :, 0:1], 0.0)
            nc.vector.memset(drTp[:, :, H + 1 : H + 2], 0.0)
            nc.vector.memset(fvA[:, :, 0:PAD1], BIGF)
            nc.vector.memset(fvA[:, :, PAD1 + H : FW], BIGF)
            nc.vector.memset(fvB[:, :, 0:1], BIGF)
            nc.vector.memset(fvB[:, :, FW - 1 : FW], BIGF)
            nc.vector.memset(g2p[:, :, 0:KH], BIG2)
            nc.vector.memset(g2p[:, :, KH + W : GW], BIG2)
            nc.vector.memset(outb[:, 3:4], 0.0)
            nc.vector.memset(warm[:, 0:1], 1.0).then_inc(w_sem, 1)

            vector.wait_ge(dma_sem, 16)
            # complement (padded); horizontal dilation
            nc.vector.tensor_scalar(nbp[:, :, 1 : W + 1], tgt, 0.5, None, Alu.is_le).then_inc(dve_sem, 1)  # d=1
            nc.vector.tensor_tensor(t1[:], nbp[:, :, 0:W], nbp[:, :, 2 : W + 2], Alu.max)
            nc.vector.tensor_tensor(dr[:], t1[:], nbp[:, :, 1 : W + 1], Alu.max).then_inc(dve_sem, 1)  # d=2

            # dr transpose blocks: copy wb0 pair as soon as it lands
            vector.wait_ge(pe_sem, 6)
            nc.vector.tensor_copy(drTp[:, 0, 1 : H + 1], ps_d[:, 0:2, :])
            vector.wait_ge(pe_sem, 8)
            nc.vector.tensor_copy(drTp[:, 1, 1 : H + 1], ps_d[:, 2:4, :])
            # vertical dilation + boundary (bound = dilated - complement)
            nc.vector.tensor_tensor(t2[:], drTp[:, :, 0:H], drTp[:, :, 2 : H + 2], Alu.max)
            nc.vector.tensor_tensor(dT[:], t2[:], drTp[:, :, 1 : H + 1], Alu.max)
            vector.wait_ge(act_sem, 1)
            nc.vector.tensor_tensor(boundT[:], dT[:], bT[:], Alu.subtract)
            nc.vector.tensor_scalar(
                fvA[:, :, PAD1 : PAD1 + H], boundT[:], -BIGF, BIGF, Alu.mult, Alu.add
            )
            # vertical L1 distance by log-doubling (window 1+2 = 3)
            nc.vector.tensor_tensor(
                tmpd[:, :, 1 : FW - 1], fvA[:, :, 0 : FW - 2], fvA[:, :, 2:FW], Alu.min
            )
            nc.vector.scalar_tensor_tensor(
                out=fvB[:, :, 1 : FW - 1], in0=tmpd[:, :, 1 : FW - 1], scalar=1.0,
                in1=fvA[:, :, 1 : FW - 1], op0=Alu.add, op1=Alu.min,
            )
            nc.vector.tensor_tensor(
                tmpd[:, :, 2 : FW - 2], fvB[:, :, 0 : FW - 4], fvB[:, :, 4:FW], Alu.min
            )
            nc.vector.scalar_tensor_tensor(
                out=fvC[:, :, 2 : FW - 2], in0=tmpd[:, :, 2 : FW - 2], scalar=2.0,
                in1=fvB[:, :, 2 : FW - 2], op0=Alu.add, op1=Alu.min,
            )
            # square the vertical distance, per W-chunk so PE can start early
            for wb in range(C):
                nc.vector.tensor_tensor(
                    g2T[:, wb, :],
                    fvC[:, wb, PAD1 : PAD1 + H],
                    fvC[:, wb, PAD1 : PAD1 + H],
                    Alu.mult,
                ).then_inc(dve_sem, 1)  # d=3 (wb=0), d=4 (wb=1)
            # diff in the g2-transpose gap (sigmoid ready: a>=2)
            vector.wait_ge(act_sem, 2)
            nc.vector.tensor_tensor(diff[:], sg[:], tgt, Alu.subtract)

            # single combined copy of the 4 g2 transpose blocks
            vector.wait_ge(pe_sem, 12)
            nc.vector.tensor_copy(g2p[:, :, KH : KH + W], psg[:, :, :])
            # phase 2: parabola min over |u| <= 3
            prev = None
            for u in range(1, KH + 1):
                in0 = g2p[:, :, KH - u : KH - u + W]
                in1 = g2p[:, :, KH + u : KH + u + W]
                nc.vector.tensor_tensor(p2tmp[:], in0, in1, Alu.min)
                base = g2p[:, :, KH : KH + W] if prev is None else prev[:]
                ins = nc.vector.scalar_tensor_tensor(
                    out=p2acc[u - 1][:], in0=p2tmp[:], scalar=float(u * u), in1=base,
                    op0=Alu.add, op1=Alu.min,
                )
                prev = p2acc[u - 1]
            d2 = prev
            ins.then_inc(dve_sem, 1)  # d=5 (d2 ready for ACT sqrt)
            nc.vector.tensor_reduce(
                out=outb[:, 1:3], in_=d2[:], axis=mybir.AxisListType.X, op=Alu.max
            )
            # weighted L1: |dist*diff| summed (dist >= 0)
            vector.wait_ge(act_sem, 3)
            nc.vector.tensor_tensor(junk[:], dist[:], diff[:], Alu.mult)
            nc.vector.tensor_reduce(
                out=outb[:, 0:1], in_=junk[:], axis=mybir.AxisListType.XY, op=Alu.add,
                apply_absolute_value=True,
            ).then_inc(dve_sem, 1)  # d=6 (outb complete)

    return nc


_NC_CACHE = {}


def _get_nc():
    if "nc" not in _NC_CACHE:
        _NC_CACHE["nc"] = _build_nc()
    return _NC_CACHE["nc"]


def _pack_input(tgt_i, prd_i):
    # [P, 4*W]: per partition p -> tgt rows p, p+128; pred rows p, p+128
    return np.concatenate([tgt_i[:P], tgt_i[P:], prd_i[:P], prd_i[P:]], axis=1)


# ---------- exact numpy fallback (pathological images only) ----------

def _reference_image_np(t, p):
    """Exact replica of the jax reference for one image, in numpy fp32."""
    b = (t > 0.5).astype(np.float32)
    if not (b > 0).any():
        return 0.0
    v = b.copy()
    v[1:] = np.minimum(v[1:], b[:-1])
    v[:-1] = np.minimum(v[:-1], b[1:])
    er = v.copy()
    er[:, 1:] = np.minimum(er[:, 1:], v[:, :-1])
    er[:, :-1] = np.minimum(er[:, :-1], v[:, 1:])
    bound = b - er
    if bound.sum() == 0:
        bound = b
    feat = bound > 0.5
    BIGV = np.float32(1e6)
    c = np.full(W, BIGV, np.float32)
    d_fwd = np.empty((H, W), np.float32)
    for i in range(H):
        c = np.where(feat[i], np.float32(0.0), c + 1)
        d_fwd[i] = c
    c = np.full(W, BIGV, np.float32)
    d_bwd = np.empty((H, W), np.float32)
    for i in range(H - 1, -1, -1):
        c = np.where(feat[i], np.float32(0.0), c + 1)
        d_bwd[i] = c
    g = np.minimum(d_fwd, d_bwd)
    j = np.arange(W, dtype=np.float32)
    d2 = np.empty((H, W), np.float32)
    for i in range(H):
        d2[i] = np.min(g[i][None, :] ** 2 + (j[:, None] - j[None, :]) ** 2, axis=-1)
    dist = np.sqrt(d2)
    m = dist.max()
    if m > 0:
        dist = dist / (m + np.float32(1e-8))
    sgm = 1.0 / (1.0 + np.exp(-p.astype(np.float64)))
    return float(np.mean(dist * np.abs(sgm - t)))


def _bound_empty(t):
    """True if erosion removes every boundary pixel (reference falls back)."""
    b = (t > 0.5).astype(np.float32)
    v = b.copy()
    v[1:] = np.minimum(v[1:], b[:-1])
    v[:-1] = np.minimum(v[:-1], b[1:])
    er = v.copy()
    er[:, 1:] = np.minimum(er[:, 1:], v[:, :-1])
    er[:, :-1] = np.minimum(er[:, :-1], v[:, 1:])
    return (b - er).sum() == 0


# ---------- public entry point ----------

def kernel(pred_logits: np.ndarray, target: np.ndarray) -> np.ndarray:
    global LAST_RESULTS
    from concourse.bass_utils import run_bass_kernel_spmd

    pred = np.ascontiguousarray(np.asarray(pred_logits, np.float32)[:, 0])
    tgt = np.ascontiguousarray(np.asarray(target, np.float32)[:, 0])
    B = pred.shape[0]
    assert pred.shape == (B, H, W) and tgt.shape == (B, H, W)
    assert B == 8, f"kernel is built for batch 8, got {B}"

    nc = _get_nc()
    in_maps = [{"inp": _pack_input(tgt[i], pred[i])} for i in range(B)]
    trace = bool(int(os.environ.get("KERNEL_TRACE", "0")))
    res = run_bass_kernel_spmd(nc, in_maps, core_ids=list(range(B)), trace=trace)
    LAST_RESULTS = res

    total = 0.0
    for i in range(B):
        o = np.asarray(res.results[i]["out"], np.float32)  # [128, 4]
        if not (tgt[i] > 0.5).any():
            continue  # empty mask: reference skips (loss 0)
        m2 = float(o[:, 1:3].max())
        if m2 > float(KH * KH) or _bound_empty(tgt[i]):
            # windowed EDT not provably exact for this image -> exact path
            total += _reference_image_np(tgt[i], pred[i])
            continue
        S = float(o[:, 0].sum(dtype=np.float64))
        m = np.float32(np.sqrt(np.float32(m2)))
        denom = float(m + np.float32(1e-8)) if m > 0 else 1.0
        total += (S / denom) / float(H * W)
    return np.float32(total / max(B, 1))


# revision 7
# speedup vs baseline: 1.0045x; 1.0035x over previous
"""Trainium2 Bass kernel for nn_BoundaryLoss (boundary EDT + weighted L1 loss).

Strategy (pure data parallel, 1 image per NeuronCore, 8 cores):
  Per image on device:
    comp    = target <= 0.5                      (complement of binary mask)
    dil     = dilate3x3(comp)  (horiz in row-major, vert after PE transpose)
    boundT  = dil - comp  (= binary AND dilated-complement, transposed)
    d2      = windowed exact EDT: vertical L1 via log-doubling (window 3),
              horizontal parabola min over |u| <= 3
    outputs per partition: sum(|sqrt(d2) * (sigmoid(pred)-target)|), max(d2)
  Host: final 128-way reduction per image + normalization + batch mean.

Windowed EDT exactness: windowed d2 >= true d2 always, with equality
guaranteed when max(windowed d2) <= KH^2 (KH = 3).  The device returns
max(d2); the host verifies the bound and falls back to an exact numpy
path for any image that fails it (never on dense masks).

vs the previous version:
  - no identity DMA: ident generated on-chip (gpsimd iota + is_eq) in the
    dead time before the input lands
  - pred queued BEHIND tgt on the same Sync HWDGE queue, so the critical
    tgt DMA has the DMA engines to itself (~1.9us earlier compute start)
  - b01/is_gt dropped: boundT = dT - nbpT (dilation minus complement)
  - all 8 stage-A transposes land in ONE 2KB PSUM bank; one ACT copy
    moves the 4 nbp blocks, one DVE copy moves the 4 dr blocks
    (replaces 8 separate per-block copies); same single-copy trick for
    the 4 g2 blocks
  - bf16 diff/dist/junk tail (dist*diff in bf16, |.| folded into reduce)
"""

import os
from contextlib import ExitStack

import numpy as np

H = 256
W = 256
P = 128
C = 2  # partition chunks per image (H = C * P)
KH = 2  # phase-2 horizontal window
# exactness: windowed d2 is exact whenever m2 <= M2_THRESH: if true d2 <= 8
# then the optimal offset has dj^2 <= 8 -> |dj| <= 2 = KH (and |di| <= 2 <= 3
# = phase-1 window), so every optimal candidate is inside the window.
M2_THRESH = 8.0
BIGF = 16384.0  # phase-1 sentinel (bf16-exact; BIGF + small stays BIGF in bf16)
BIG2 = 3.0e8  # phase-2 border pad, > (BIGF+3)^2
PAD1 = 4  # phase-1 doubling pads (window 1+2; 4 keeps slices 4B-aligned)
FW = H + 2 * PAD1
GW = W + 2 * KH

LAST_RESULTS = None  # BassKernelResults of the most recent device run


def _build_nc():
    import concourse.bass as bass
    import concourse.mybir as mybir

    bf16 = mybir.dt.bfloat16
    f32 = mybir.dt.float32
    i32 = mybir.dt.int32
    Alu = mybir.AluOpType
    Act = mybir.ActivationFunctionType

    nc = bass.Bass(detect_race_conditions=False)
    inp_d = nc.dram_tensor("inp", [P, 4 * W], f32, kind="ExternalInput")
    out_d = nc.dram_tensor("out", [P, 4], f32, kind="ExternalOutput")

    ctx = ExitStack()
    sb = lambda name, shape, dt: ctx.enter_context(nc.sbuf_tensor(name, shape, dt))

    with ctx:
        inp = sb("inp_t", [P, 4, W], f32)
        tgt = inp[:, 0:C, :]
        prd = inp[:, C : 2 * C, :]
        ident = sb("ident", [P, P], bf16)
        iot = sb("iot", [P, P], i32)
        nbp = sb("nbp", [P, C, W + 2], bf16)
        t1 = sb("t1", [P, C, W], bf16)
        dr = sb("dr", [P, C, W], bf16)
        bT = sb("bT", [P, C, H], bf16)       # transposed complement
        drTp = sb("drTp", [P, C, H + 2], bf16)
        t2 = sb("t2", [P, C, H], bf16)
        dT = sb("dT", [P, C, H], bf16)
        boundT = sb("boundT", [P, C, H], bf16)
        fvA = sb("fvA", [P, C, FW], bf16)
        fvB = sb("fvB", [P, C, FW], bf16)
        fvC = sb("fvC", [P, C, FW], bf16)
        tmpd = sb("tmpd", [P, C, FW], bf16)
        g2T = sb("g2T", [P, C, H], bf16)
        g2p = sb("g2p", [P, C, GW], bf16)
        p2tmp = sb("p2tmp", [P, C, W], bf16)
        p2acc = [sb(f"p2acc{i}", [P, C, W], bf16) for i in range(KH)]
        dist = sb("dist", [P, C, W], bf16)
        sg = sb("sg", [P, C, W], f32)
        diff = sb("diff", [P, C, W], bf16)
        junk = sb("junk", [P, C, W], bf16)
        outb = sb("outb", [P, 4], f32)
        warm = sb("warm", [P, 4], f32)
        ps_n = ctx.enter_context(nc.psum_tensor("ps_n", [P, 4, P], bf16))
        ps_d = ctx.enter_context(nc.psum_tensor("ps_d", [P, 4, P], bf16))
        psg = ctx.enter_context(nc.psum_tensor("psg", [P, 4, P], bf16))

        dma_sem = ctx.enter_context(nc.semaphore("dma_sem"))
        dma2_sem = ctx.enter_context(nc.semaphore("dma2_sem"))
        dve_sem = ctx.enter_context(nc.semaphore("dve_sem"))
        act_sem = ctx.enter_context(nc.semaphore("act_sem"))
        pe_sem = ctx.enter_context(nc.semaphore("pe_sem"))
        gp_sem = ctx.enter_context(nc.semaphore("gp_sem"))
        w_sem = ctx.enter_context(nc.semaphore("w_sem"))

        block = ctx.enter_context(nc.Block(no_gpsimd_drain=True))

        @block.sync
        def _(sync: "bass.BassEngine"):
            # tgt first (critical), pred queued right behind on the same queue
            sync.dma_start(out=inp[:, 0:C, :], in_=inp_d[:, 0 : C * W]).then_inc(dma_sem, 16)
            sync.dma_start(out=inp[:, C : 2 * C, :], in_=inp_d[:, C * W : 2 * C * W]).then_inc(dma2_sem, 16)
            # out DMA (after the DVE chain fully wrote outb)
            sync.wait_ge(dve_sem, 6)
            sync.dma_start(out=out_d[:], in_=outb[:]).then_inc(dma_sem, 16)
            sync.wait_ge(dma_sem, 32)

        @block.gpsimd
        def _(gpsimd: "bass.BassEngine"):
            # identity matrix on-chip: iota(j - p) == 0
            nc.gpsimd.iota(iot[:], [[1, P]], base=0, channel_multiplier=-1)
            nc.gpsimd.tensor_scalar(ident[:], iot[:], 0, None, Alu.is_equal).then_inc(gp_sem, 1)

        @block.scalar
        def _(scalar: "bass.BassEngine"):
            # warm the sigmoid/copy table while the input DMA flies
            scalar.wait_ge(w_sem, 1)
            nc.scalar.activation(warm[:, 1:2], warm[:, 0:1], Act.Sigmoid)
            nc.scalar.copy(warm[:, 1:2], warm[:, 0:1])
            # bT: all 4 nbp transpose blocks in one copy from PSUM bank
            scalar.wait_ge(pe_sem, 4)
            nc.scalar.copy(bT[:], ps_n[:, :, :]).then_inc(act_sem, 1)  # a=1
            # sigmoid (pred lands right behind tgt on the sync queue)
            scalar.wait_ge(dma2_sem, 16)
            nc.scalar.activation(sg[:], prd, Act.Sigmoid).then_inc(act_sem, 1)  # a=2
            # re-warm the sqrt table during phase 2 (sigmoid load evicted it)
            scalar.wait_ge(dve_sem, 4)
            nc.scalar.sqrt(warm[:, 2:3], warm[:, 0:1])
            # sqrt(d2) (after DVE phase 2: d=5)
            scalar.wait_ge(dve_sem, 5)
            nc.scalar.sqrt(dist[:], p2acc[KH - 1][:]).then_inc(act_sem, 1)  # a=3

        @block.tensor
        def _(tensor: "bass.BassEngine"):
            # nbp transposes (DVE d=1) using on-chip ident (gp=1)
            tensor.wait_ge(gp_sem, 1)
            tensor.wait_ge(dve_sem, 1)
            for wb in range(C):
                for hc in range(C):
                    nc.tensor.transpose(
                        ps_n[:, wb * C + hc, :],
                        nbp[:, hc, 1 + wb * P : 1 + (wb + 1) * P],
                        ident[:],
                    ).then_inc(pe_sem, 1)  # pe 1..4
            # dr transposes (DVE d=2)
            tensor.wait_ge(dve_sem, 2)
            for wb in range(C):
                for hc in range(C):
                    nc.tensor.transpose(
                        ps_d[:, wb * C + hc, :],
                        dr[:, hc, wb * P : (wb + 1) * P],
                        ident[:],
                    ).then_inc(pe_sem, 1)  # pe 5..8
            # g2 transposes back to row-major, per W-chunk as squares land
            for wb in range(C):
                tensor.wait_ge(dve_sem, 3 + wb)
                for hc in range(C):
                    nc.tensor.transpose(
                        psg[:, hc * C + wb, :],
                        g2T[:, wb, hc * P : (hc + 1) * P],
                        ident[:],
                    ).then_inc(pe_sem, 1)  # pe 9..12

        @block.vector
        def _(vector: "bass.BassEngine"):
            # data-independent pad memsets first (no waits)
            nc.vector.memset(nbp[:, :, 0:1], 0.0)
            nc.vector.memset(nbp[:, :, W + 1 : W + 2], 0.0)
            nc.vector.memset(drTp[:, :, 0:1], 0.0)
            nc.vector.memset(drTp[:, :, H + 1 : H + 2], 0.0)
            nc.vector.memset(fvA[:, :, 0:PAD1], BIGF)
            nc.vector.memset(fvA[:, :, PAD1 + H : FW], BIGF)
            nc.vector.memset(fvB[:, :, 0:1], BIGF)
            nc.vector.memset(fvB[:, :, FW - 1 : FW], BIGF)
            nc.vector.memset(g2p[:, :, 0:KH], BIG2)
            nc.vector.memset(g2p[:, :, KH + W : GW], BIG2)
            nc.vector.memset(outb[:, 3:4], 0.0)
            nc.vector.memset(warm[:, 0:1], 1.0).then_inc(w_sem, 1)

            vector.wait_ge(dma_sem, 16)
            # complement (padded); horizontal dilation
            nc.vector.tensor_scalar(nbp[:, :, 1 : W + 1], tgt, 0.5, None, Alu.is_le).then_inc(dve_sem, 1)  # d=1
            nc.vector.tensor_tensor(t1[:], nbp[:, :, 0:W], nbp[:, :, 2 : W + 2], Alu.max)
            nc.vector.tensor_tensor(dr[:], t1[:], nbp[:, :, 1 : W + 1], Alu.max).then_inc(dve_sem, 1)  # d=2

            # dr transpose blocks: copy wb0 pair as soon as it lands
            vector.wait_ge(pe_sem, 6)
            nc.vector.tensor_copy(drTp[:, 0, 1 : H + 1], ps_d[:, 0:2, :])
            vector.wait_ge(pe_sem, 8)
            nc.vector.tensor_copy(drTp[:, 1, 1 : H + 1], ps_d[:, 2:4, :])
            # vertical dilation + boundary (bound = dilated - complement)
            nc.vector.tensor_tensor(t2[:], drTp[:, :, 0:H], drTp[:, :, 2 : H + 2], Alu.max)
            nc.vector.tensor_tensor(dT[:], t2[:], drTp[:, :, 1 : H + 1], Alu.max)
            vector.wait_ge(act_sem, 1)
            nc.vector.tensor_tensor(boundT[:], dT[:], bT[:], Alu.subtract)
            nc.vector.tensor_scalar(
                fvA[:, :, PAD1 : PAD1 + H], boundT[:], -BIGF, BIGF, Alu.mult, Alu.add
            )
            # vertical L1 distance by log-doubling (window 1+2 = 3)
            nc.vector.tensor_tensor(
                tmpd[:, :, 1 : FW - 1], fvA[:, :, 0 : FW - 2], fvA[:, :, 2:FW], Alu.min
            )
            nc.vector.scalar_tensor_tensor(
                out=fvB[:, :, 1 : FW - 1], in0=tmpd[:, :, 1 : FW - 1], scalar=1.0,
                in1=fvA[:, :, 1 : FW - 1], op0=Alu.add, op1=Alu.min,
            )
            nc.vector.tensor_tensor(
                tmpd[:, :, 2 : FW - 2], fvB[:, :, 0 : FW - 4], fvB[:, :, 4:FW], Alu.min
            )
            nc.vector.scalar_tensor_tensor(
                out=fvC[:, :, 2 : FW - 2], in0=tmpd[:, :, 2 : FW - 2], scalar=2.0,
                in1=fvB[:, :, 2 : FW - 2], op0=Alu.add, op1=Alu.min,
            )
            # square the vertical distance, per W-chunk so PE can start early
            for wb in range(C):
                nc.vector.tensor_tensor(
                    g2T[:, wb, :],
                    fvC[:, wb, PAD1 : PAD1 + H],
                    fvC[:, wb, PAD1 : PAD1 + H],
                    Alu.mult,
                ).then_inc(dve_sem, 1)  # d=3 (wb=0), d=4 (wb=1)
            # diff in the g2-transpose gap (sigmoid ready: a>=2)
            vector.wait_ge(act_sem, 2)
            nc.vector.tensor_tensor(diff[:], sg[:], tgt, Alu.subtract)

            # single combined copy of the 4 g2 transpose blocks
            vector.wait_ge(pe_sem, 12)
            nc.vector.tensor_copy(g2p[:, :, KH : KH + W], psg[:, :, :])
            # phase 2: parabola min over |u| <= 3
            prev = None
            for u in range(1, KH + 1):
                in0 = g2p[:, :, KH - u : KH - u + W]
                in1 = g2p[:, :, KH + u : KH + u + W]
                nc.vector.tensor_tensor(p2tmp[:], in0, in1, Alu.min)
                base = g2p[:, :, KH : KH + W] if prev is None else prev[:]
                ins = nc.vector.scalar_tensor_tensor(
                    out=p2acc[u - 1][:], in0=p2tmp[:], scalar=float(u * u), in1=base,
                    op0=Alu.add, op1=Alu.min,
                )
                prev = p2acc[u - 1]
            d2 = prev
            ins.then_inc(dve_sem, 1)  # d=5 (d2 ready for ACT sqrt)
            nc.vector.tensor_reduce(
                out=outb[:, 1:3], in_=d2[:], axis=mybir.AxisListType.X, op=Alu.max
            )
            # weighted L1: |dist*diff| summed (dist >= 0)
            vector.wait_ge(act_sem, 3)
            nc.vector.tensor_tensor(junk[:], dist[:], diff[:], Alu.mult)
            nc.vector.tensor_reduce(
                out=outb[:, 0:1], in_=junk[:], axis=mybir.AxisListType.XY, op=Alu.add,
                apply_absolute_value=True,
            ).then_inc(dve_sem, 1)  # d=6 (outb complete)

    return nc


_NC_CACHE = {}


def _get_nc():
    if "nc" not in _NC_CACHE:
        _NC_CACHE["nc"] = _build_nc()
    return _NC_CACHE["nc"]


def _pack_input(tgt_i, prd_i):
    # [P, 4*W]: per partition p -> tgt rows p, p+128; pred rows p, p+128
    return np.concatenate([tgt_i[:P], tgt_i[P:], prd_i[:P], prd_i[P:]], axis=1)


# ---------- exact numpy fallback (pathological images only) ----------

def _reference_image_np(t, p):
    """Exact replica of the jax reference for one image, in numpy fp32."""
    b = (t > 0.5).astype(np.float32)
    if not (b > 0).any():
        return 0.0
    v = b.copy()
    v[1:] = np.minimum(v[1:], b[:-1])
    v[:-1] = np.minimum(v[:-1], b[1:])
    er = v.copy()
    er[:, 1:] = np.minimum(er[:, 1:], v[:, :-1])
    er[:, :-1] = np.minimum(er[:, :-1], v[:, 1:])
    bound = b - er
    if bound.sum() == 0:
        bound = b
    feat = bound > 0.5
    BIGV = np.float32(1e6)
    c = np.full(W, BIGV, np.float32)
    d_fwd = np.empty((H, W), np.float32)
    for i in range(H):
        c = np.where(feat[i], np.float32(0.0), c + 1)
        d_fwd[i] = c
    c = np.full(W, BIGV, np.float32)
    d_bwd = np.empty((H, W), np.float32)
    for i in range(H - 1, -1, -1):
        c = np.where(feat[i], np.float32(0.0), c + 1)
        d_bwd[i] = c
    g = np.minimum(d_fwd, d_bwd)
    j = np.arange(W, dtype=np.float32)
    d2 = np.empty((H, W), np.float32)
    for i in range(H):
        d2[i] = np.min(g[i][None, :] ** 2 + (j[:, None] - j[None, :]) ** 2, axis=-1)
    dist = np.sqrt(d2)
    m = dist.max()
    if m > 0:
        dist = dist / (m + np.float32(1e-8))
    sgm = 1.0 / (1.0 + np.exp(-p.astype(np.float64)))
    return float(np.mean(dist * np.abs(sgm - t)))


def _bound_empty(t):
    """True if erosion removes every boundary pixel (reference falls back)."""
    b = (t > 0.5).astype(np.float32)
    v = b.copy()
    v[1:] = np.minimum(v[1:], b[:-1])
    v[:-1] = np.minimum(v[:-1], b[1:])
    er = v.copy()
    er[:, 1:] = np.minimum(er[:, 1:], v[:, :-1])
    er[:, :-1] = np.minimum(er[:, :-1], v[:, 1:])
    return (b - er).sum() == 0


# ---------- public entry point ----------

def kernel(pred_logits: np.ndarray, target: np.ndarray) -> np.ndarray:
    global LAST_RESULTS
    from concourse.bass_utils import run_bass_kernel_spmd

    pred = np.ascontiguousarray(np.asarray(pred_logits, np.float32)[:, 0])
    tgt = np.ascontiguousarray(np.asarray(target, np.float32)[:, 0])
    B = pred.shape[0]
    assert pred.shape == (B, H, W) and tgt.shape == (B, H, W)
    assert B == 8, f"kernel is built for batch 8, got {B}"

    nc = _get_nc()
    in_maps = [{"inp": _pack_input(tgt[i], pred[i])} for i in range(B)]
    trace = bool(int(os.environ.get("KERNEL_TRACE", "0")))
    res = run_bass_kernel_spmd(nc, in_maps, core_ids=list(range(B)), trace=trace)
    LAST_RESULTS = res

    total = 0.0
    for i in range(B):
        o = np.asarray(res.results[i]["out"], np.float32)  # [128, 4]
        if not (tgt[i] > 0.5).any():
            continue  # empty mask: reference skips (loss 0)
        m2 = float(o[:, 1:3].max())
        if m2 > M2_THRESH or _bound_empty(tgt[i]):
            # windowed EDT not provably exact for this image -> exact path
            total += _reference_image_np(tgt[i], pred[i])
            continue
        S = float(o[:, 0].sum(dtype=np.float64))
        m = np.float32(np.sqrt(np.float32(m2)))
        denom = float(m + np.float32(1e-8)) if m > 0 else 1.0
        total += (S / denom) / float(H * W)
    return np.float32(total / max(B, 1))


# revision 8
# speedup vs baseline: 1.0573x; 1.0526x over previous
"""Trainium2 Bass kernel for nn_BoundaryLoss (boundary EDT + weighted L1 loss).

Strategy (pure data parallel, 1 image per NeuronCore, 8 cores):
  Per image on device:
    comp    = target <= 0.5                      (complement of binary mask)
    dil     = dilate3x3(comp)  (horiz in row-major, vert after PE transpose)
    boundT  = dil - comp  (= binary AND dilated-complement, transposed)
    d2      = windowed exact EDT: vertical L1 via log-doubling (window 3),
              horizontal parabola min over |u| <= 3
    outputs per partition: sum(|sqrt(d2) * (sigmoid(pred)-target)|), max(d2)
  Host: final 128-way reduction per image + normalization + batch mean.

Windowed EDT exactness: windowed d2 >= true d2 always, with equality
guaranteed when max(windowed d2) <= KH^2 (KH = 3).  The device returns
max(d2); the host verifies the bound and falls back to an exact numpy
path for any image that fails it (never on dense masks).

vs the previous version:
  - no identity DMA: ident generated on-chip (gpsimd iota + is_eq) in the
    dead time before the input lands
  - pred queued BEHIND tgt on the same Sync HWDGE queue, so the critical
    tgt DMA has the DMA engines to itself (~1.9us earlier compute start)
  - b01/is_gt dropped: boundT = dT - nbpT (dilation minus complement)
  - all 8 stage-A transposes land in ONE 2KB PSUM bank; one ACT copy
    moves the 4 nbp blocks, one DVE copy moves the 4 dr blocks
    (replaces 8 separate per-block copies); same single-copy trick for
    the 4 g2 blocks
  - bf16 diff/dist/junk tail (dist*diff in bf16, |.| folded into reduce)
"""

import os
from contextlib import ExitStack

import numpy as np

H = 256
W = 256
P = 128
C = 2  # partition chunks per image (H = C * P)
KH = 2  # phase-2 horizontal window
# exactness: windowed d2 is exact whenever m2 <= M2_THRESH: if true d2 <= 8
# then the optimal offset has dj^2 <= 8 -> |dj| <= 2 = KH (and |di| <= 2 <= 3
# = phase-1 window), so every optimal candidate is inside the window.
M2_THRESH = 8.0
BIGF = 16384.0  # phase-1 sentinel (bf16-exact; BIGF + small stays BIGF in bf16)
BIG2 = 3.0e8  # phase-2 border pad, > (BIGF+3)^2
PAD1 = 4  # phase-1 doubling pads (window 1+2; 4 keeps slices 4B-aligned)
FW = H + 2 * PAD1
GW = W + 2 * KH

LAST_RESULTS = None  # BassKernelResults of the most recent device run


def _build_nc():
    import concourse.bass as bass
    import concourse.mybir as mybir

    bf16 = mybir.dt.bfloat16
    f32 = mybir.dt.float32
    i32 = mybir.dt.int32
    Alu = mybir.AluOpType
    Act = mybir.ActivationFunctionType

    nc = bass.Bass(detect_race_conditions=False)
    inp_d = nc.dram_tensor("inp", [P, 4 * W], bf16, kind="ExternalInput")
    out_d = nc.dram_tensor("out", [P, 4], f32, kind="ExternalOutput")

    ctx = ExitStack()
    sb = lambda name, shape, dt: ctx.enter_context(nc.sbuf_tensor(name, shape, dt))

    with ctx:
        inp = sb("inp_t", [P, 4, W], bf16)
        tgt = inp[:, 0:C, :]
        prd = inp[:, C : 2 * C, :]
        ident = sb("ident", [P, P], bf16)
        iot = sb("iot", [P, P], i32)
        nbp = sb("nbp", [P, C, W + 2], bf16)
        t1 = sb("t1", [P, C, W], bf16)
        dr = sb("dr", [P, C, W], bf16)
        bT = sb("bT", [P, C, H], bf16)       # transposed complement
        drTp = sb("drTp", [P, C, H + 2], bf16)
        t2 = sb("t2", [P, C, H], bf16)
        dT = sb("dT", [P, C, H], bf16)
        boundT = sb("boundT", [P, C, H], bf16)
        fvA = sb("fvA", [P, C, FW], bf16)
        fvB = sb("fvB", [P, C, FW], bf16)
        fvC = sb("fvC", [P, C, FW], bf16)
        tmpd = sb("tmpd", [P, C, FW], bf16)
        g2T = sb("g2T", [P, C, H], bf16)
        g2p = sb("g2p", [P, C, GW], bf16)
        p2tmp = sb("p2tmp", [P, C, W], bf16)
        p2acc = [sb(f"p2acc{i}", [P, C, W], bf16) for i in range(KH)]
        dist = sb("dist", [P, C, W], bf16)
        sg = sb("sg", [P, C, W], f32)
        diff = sb("diff", [P, C, W], bf16)
        junk = sb("junk", [P, C, W], bf16)
        outb = sb("outb", [P, 4], f32)
        warm = sb("warm", [P, 4], f32)
        ps_n = ctx.enter_context(nc.psum_tensor("ps_n", [P, 4, P], bf16))
        ps_d = ctx.enter_context(nc.psum_tensor("ps_d", [P, 4, P], bf16))
        psg = ctx.enter_context(nc.psum_tensor("psg", [P, 4, P], bf16))

        dma_sem = ctx.enter_context(nc.semaphore("dma_sem"))
        dma2_sem = ctx.enter_context(nc.semaphore("dma2_sem"))
        dve_sem = ctx.enter_context(nc.semaphore("dve_sem"))
        act_sem = ctx.enter_context(nc.semaphore("act_sem"))
        pe_sem = ctx.enter_context(nc.semaphore("pe_sem"))
        gp_sem = ctx.enter_context(nc.semaphore("gp_sem"))
        w_sem = ctx.enter_context(nc.semaphore("w_sem"))

        block = ctx.enter_context(nc.Block(no_gpsimd_drain=True))

        @block.sync
        def _(sync: "bass.BassEngine"):
            # tgt first (critical), pred queued right behind on the same queue
            sync.dma_start(out=inp[:, 0:C, :], in_=inp_d[:, 0 : C * W]).then_inc(dma_sem, 16)
            sync.dma_start(out=inp[:, C : 2 * C, :], in_=inp_d[:, C * W : 2 * C * W]).then_inc(dma2_sem, 16)
            # out DMA (after the DVE chain fully wrote outb)
            sync.wait_ge(dve_sem, 6)
            sync.dma_start(out=out_d[:], in_=outb[:]).then_inc(dma_sem, 16)
            sync.wait_ge(dma_sem, 32)

        @block.gpsimd
        def _(gpsimd: "bass.BassEngine"):
            # identity matrix on-chip: iota(j - p) == 0
            nc.gpsimd.iota(iot[:], [[1, P]], base=0, channel_multiplier=-1)
            nc.gpsimd.tensor_scalar(ident[:], iot[:], 0, None, Alu.is_equal).then_inc(gp_sem, 1)

        @block.scalar
        def _(scalar: "bass.BassEngine"):
            # warm the sigmoid/copy table while the input DMA flies
            scalar.wait_ge(w_sem, 1)
            nc.scalar.activation(warm[:, 1:2], warm[:, 0:1], Act.Sigmoid)
            nc.scalar.copy(warm[:, 1:2], warm[:, 0:1])
            # bT: all 4 nbp transpose blocks in one copy from PSUM bank
            scalar.wait_ge(pe_sem, 4)
            nc.scalar.copy(bT[:], ps_n[:, :, :]).then_inc(act_sem, 1)  # a=1
            # sigmoid (pred lands right behind tgt on the sync queue)
            scalar.wait_ge(dma2_sem, 16)
            nc.scalar.activation(sg[:], prd, Act.Sigmoid).then_inc(act_sem, 1)  # a=2
            # re-warm the sqrt table during phase 2 (sigmoid load evicted it)
            scalar.wait_ge(dve_sem, 4)
            nc.scalar.sqrt(warm[:, 2:3], warm[:, 0:1])
            # sqrt(d2) (after DVE phase 2: d=5)
            scalar.wait_ge(dve_sem, 5)
            nc.scalar.sqrt(dist[:], p2acc[KH - 1][:]).then_inc(act_sem, 1)  # a=3

        @block.tensor
        def _(tensor: "bass.BassEngine"):
            # nbp transposes (DVE d=1) using on-chip ident (gp=1)
            tensor.wait_ge(gp_sem, 1)
            tensor.wait_ge(dve_sem, 1)
            for wb in range(C):
                for hc in range(C):
                    nc.tensor.transpose(
                        ps_n[:, wb * C + hc, :],
                        nbp[:, hc, 1 + wb * P : 1 + (wb + 1) * P],
                        ident[:],
                    ).then_inc(pe_sem, 1)  # pe 1..4
            # dr transposes (DVE d=2)
            tensor.wait_ge(dve_sem, 2)
            for wb in range(C):
                for hc in range(C):
                    nc.tensor.transpose(
                        ps_d[:, wb * C + hc, :],
                        dr[:, hc, wb * P : (wb + 1) * P],
                        ident[:],
                    ).then_inc(pe_sem, 1)  # pe 5..8
            # g2 transposes back to row-major, per W-chunk as squares land
            for wb in range(C):
                tensor.wait_ge(dve_sem, 3 + wb)
                for hc in range(C):
                    nc.tensor.transpose(
                        psg[:, hc * C + wb, :],
                        g2T[:, wb, hc * P : (hc + 1) * P],
                        ident[:],
                    ).then_inc(pe_sem, 1)  # pe 9..12

        @block.vector
        def _(vector: "bass.BassEngine"):
            # data-independent pad memsets first (no waits)
            nc.vector.memset(nbp[:, :, 0:1], 0.0)
            nc.vector.memset(nbp[:, :, W + 1 : W + 2], 0.0)
            nc.vector.memset(drTp[:, :, 0:1], 0.0)
            nc.vector.memset(drTp[:, :, H + 1 : H + 2], 0.0)
            nc.vector.memset(fvA[:, :, 0:PAD1], BIGF)
            nc.vector.memset(fvA[:, :, PAD1 + H : FW], BIGF)
            nc.vector.memset(fvB[:, :, 0:1], BIGF)
            nc.vector.memset(fvB[:, :, FW - 1 : FW], BIGF)
            nc.vector.memset(g2p[:, :, 0:KH], BIG2)
            nc.vector.memset(g2p[:, :, KH + W : GW], BIG2)
            nc.vector.memset(outb[:, 3:4], 0.0)
            nc.vector.memset(warm[:, 0:1], 1.0).then_inc(w_sem, 1)

            vector.wait_ge(dma_sem, 16)
            # complement (padded); horizontal dilation
            nc.vector.tensor_scalar(nbp[:, :, 1 : W + 1], tgt, 0.5, None, Alu.is_le).then_inc(dve_sem, 1)  # d=1
            nc.vector.tensor_tensor(t1[:], nbp[:, :, 0:W], nbp[:, :, 2 : W + 2], Alu.max)
            nc.vector.tensor_tensor(dr[:], t1[:], nbp[:, :, 1 : W + 1], Alu.max).then_inc(dve_sem, 1)  # d=2

            # dr transpose blocks: copy wb0 pair as soon as it lands
            vector.wait_ge(pe_sem, 6)
            nc.vector.tensor_copy(drTp[:, 0, 1 : H + 1], ps_d[:, 0:2, :])
            vector.wait_ge(pe_sem, 8)
            nc.vector.tensor_copy(drTp[:, 1, 1 : H + 1], ps_d[:, 2:4, :])
            # vertical dilation + boundary (bound = dilated - complement)
            nc.vector.tensor_tensor(t2[:], drTp[:, :, 0:H], drTp[:, :, 2 : H + 2], Alu.max)
            nc.vector.tensor_tensor(dT[:], t2[:], drTp[:, :, 1 : H + 1], Alu.max)
            vector.wait_ge(act_sem, 1)
            nc.vector.tensor_tensor(boundT[:], dT[:], bT[:], Alu.subtract)
            nc.vector.tensor_scalar(
                fvA[:, :, PAD1 : PAD1 + H], boundT[:], -BIGF, BIGF, Alu.mult, Alu.add
            )
            # vertical L1 distance by log-doubling (window 1+2 = 3)
            nc.vector.tensor_tensor(
                tmpd[:, :, 1 : FW - 1], fvA[:, :, 0 : FW - 2], fvA[:, :, 2:FW], Alu.min
            )
            nc.vector.scalar_tensor_tensor(
                out=fvB[:, :, 1 : FW - 1], in0=tmpd[:, :, 1 : FW - 1], scalar=1.0,
                in1=fvA[:, :, 1 : FW - 1], op0=Alu.add, op1=Alu.min,
            )
            nc.vector.tensor_tensor(
                tmpd[:, :, 2 : FW - 2], fvB[:, :, 0 : FW - 4], fvB[:, :, 4:FW], Alu.min
            )
            nc.vector.scalar_tensor_tensor(
                out=fvC[:, :, 2 : FW - 2], in0=tmpd[:, :, 2 : FW - 2], scalar=2.0,
                in1=fvB[:, :, 2 : FW - 2], op0=Alu.add, op1=Alu.min,
            )
            # square the vertical distance, per W-chunk so PE can start early
            for wb in range(C):
                nc.vector.tensor_tensor(
                    g2T[:, wb, :],
                    fvC[:, wb, PAD1 : PAD1 + H],
                    fvC[:, wb, PAD1 : PAD1 + H],
                    Alu.mult,
                ).then_inc(dve_sem, 1)  # d=3 (wb=0), d=4 (wb=1)
            # diff in the g2-transpose gap (sigmoid ready: a>=2)
            vector.wait_ge(act_sem, 2)
            nc.vector.tensor_tensor(diff[:], sg[:], tgt, Alu.subtract)

            # single combined copy of the 4 g2 transpose blocks
            vector.wait_ge(pe_sem, 12)
            nc.vector.tensor_copy(g2p[:, :, KH : KH + W], psg[:, :, :])
            # phase 2: parabola min over |u| <= 3
            prev = None
            for u in range(1, KH + 1):
                in0 = g2p[:, :, KH - u : KH - u + W]
                in1 = g2p[:, :, KH + u : KH + u + W]
                nc.vector.tensor_tensor(p2tmp[:], in0, in1, Alu.min)
                base = g2p[:, :, KH : KH + W] if prev is None else prev[:]
                ins = nc.vector.scalar_tensor_tensor(
                    out=p2acc[u - 1][:], in0=p2tmp[:], scalar=float(u * u), in1=base,
                    op0=Alu.add, op1=Alu.min,
                )
                prev = p2acc[u - 1]
            d2 = prev
            ins.then_inc(dve_sem, 1)  # d=5 (d2 ready for ACT sqrt)
            nc.vector.tensor_reduce(
                out=outb[:, 1:3], in_=d2[:], axis=mybir.AxisListType.X, op=Alu.max
            )
            # weighted L1: |dist*diff| summed (dist >= 0)
            vector.wait_ge(act_sem, 3)
            nc.vector.tensor_tensor(junk[:], dist[:], diff[:], Alu.mult)
            nc.vector.tensor_reduce(
                out=outb[:, 0:1], in_=junk[:], axis=mybir.AxisListType.XY, op=Alu.add,
                apply_absolute_value=True,
            ).then_inc(dve_sem, 1)  # d=6 (outb complete)

    return nc


_NC_CACHE = {}


def _get_nc():
    if "nc" not in _NC_CACHE:
        _NC_CACHE["nc"] = _build_nc()
    return _NC_CACHE["nc"]


def _pack_input(tgt_i, prd_i):
    # [P, 4*W] bf16: per partition p -> tgt rows p, p+128; pred rows p, p+128
    import ml_dtypes
    packed = np.concatenate([tgt_i[:P], tgt_i[P:], prd_i[:P], prd_i[P:]], axis=1)
    return packed.astype(ml_dtypes.bfloat16)


# ---------- exact numpy fallback (pathological images only) ----------

def _reference_image_np(t, p):
    """Exact replica of the jax reference for one image, in numpy fp32."""
    b = (t > 0.5).astype(np.float32)
    if not (b > 0).any():
        return 0.0
    v = b.copy()
    v[1:] = np.minimum(v[1:], b[:-1])
    v[:-1] = np.minimum(v[:-1], b[1:])
    er = v.copy()
    er[:, 1:] = np.minimum(er[:, 1:], v[:, :-1])
    er[:, :-1] = np.minimum(er[:, :-1], v[:, 1:])
    bound = b - er
    if bound.sum() == 0:
        bound = b
    feat = bound > 0.5
    BIGV = np.float32(1e6)
    c = np.full(W, BIGV, np.float32)
    d_fwd = np.empty((H, W), np.float32)
    for i in range(H):
        c = np.where(feat[i], np.float32(0.0), c + 1)
        d_fwd[i] = c
    c = np.full(W, BIGV, np.float32)
    d_bwd = np.empty((H, W), np.float32)
    for i in range(H - 1, -1, -1):
        c = np.where(feat[i], np.float32(0.0), c + 1)
        d_bwd[i] = c
    g = np.minimum(d_fwd, d_bwd)
    j = np.arange(W, dtype=np.float32)
    d2 = np.empty((H, W), np.float32)
    for i in range(H):
        d2[i] = np.min(g[i][None, :] ** 2 + (j[:, None] - j[None, :]) ** 2, axis=-1)
    dist = np.sqrt(d2)
    m = dist.max()
    if m > 0:
        dist = dist / (m + np.float32(1e-8))
    sgm = 1.0 / (1.0 + np.exp(-p.astype(np.float64)))
    return float(np.mean(dist * np.abs(sgm - t)))


def _bound_empty(t):
    """True if erosion removes every boundary pixel (reference falls back)."""
    b = (t > 0.5).astype(np.float32)
    v = b.copy()
    v[1:] = np.minimum(v[1:], b[:-1])
    v[:-1] = np.minimum(v[:-1], b[1:])
    er = v.copy()
    er[:, 1:] = np.minimum(er[:, 1:], v[:, :-1])
    er[:, :-1] = np.minimum(er[:, :-1], v[:, 1:])
    return (b - er).sum() == 0


# ---------- public entry point ----------

def kernel(pred_logits: np.ndarray, target: np.ndarray) -> np.ndarray:
    global LAST_RESULTS
    from concourse.bass_utils import run_bass_kernel_spmd

    pred = np.ascontiguousarray(np.asarray(pred_logits, np.float32)[:, 0])
    tgt = np.ascontiguousarray(np.asarray(target, np.float32)[:, 0])
    B = pred.shape[0]
    assert pred.shape == (B, H, W) and tgt.shape == (B, H, W)
    assert B == 8, f"kernel is built for batch 8, got {B}"

    nc = _get_nc()
    in_maps = [{"inp": _pack_input(tgt[i], pred[i])} for i in range(B)]
    trace = bool(int(os.environ.get("KERNEL_TRACE", "0")))
    res = run_bass_kernel_spmd(nc, in_maps, core_ids=list(range(B)), trace=trace)
    LAST_RESULTS = res

    total = 0.0
    for i in range(B):
        o = np.asarray(res.results[i]["out"], np.float32)  # [128, 4]
        if not (tgt[i] > 0.5).any():
            continue  # empty mask: reference skips (loss 0)
        m2 = float(o[:, 1:3].max())
        if m2 > M2_THRESH or _bound_empty(tgt[i]):
            # windowed EDT not provably exact for this image -> exact path
            total += _reference_image_np(tgt[i], pred[i])
            continue
        S = float(o[:, 0].sum(dtype=np.float64))
        m = np.float32(np.sqrt(np.float32(m2)))
        denom = float(m + np.float32(1e-8)) if m > 0 else 1.0
        total += (S / denom) / float(H * W)
    return np.float32(total / max(B, 1))
